# revision 14
# baseline (speedup 1.0000x reference)
"""Trainium2 Bass kernel for batched nearest-neighbor min-distance.

Problem: for each row u of U_z [16384, 256], compute
    min_{l in L_z [8192, 256]} ||u - l||_2
Strategy (8 NeuronCores, data-parallel over rows of U_z, L_z replicated;
`pred` is unused by the reference and ignored):
  d2(u,l) = ||u||^2 + ||l||^2 - 2 u.l
v3: fp8(e4m3) DoubleRowSwInterleave matmuls (as v2) + a CUSTOM DVE uop
(MIN3_PB_ANT, registered at runtime into concourse.dve_ops) that breaks the
two-engine consumer bound:
  Per core (2048 U cols), 64 L-tiles of [128 Lrows x 2048 Ucols] fp32 PSUM.
  v2 consumed each tile with either an ACT conv (2.0us) + DVE fp16 merge
  (1.13us) or a DVE fused stt (2.26us); LP-optimal mix ~92us/core of engine
  time (both engines saturated -- measured ~90-115us slope).
  v3 consumes tiles in PAIRS: even tile -> ACT conv (bias=l2c, fp32 PSUM ->
  bf16) written STRIDE-2 into the odd slots of an interleaved buffer
  Z = [(rmin_0, conv_0), (rmin_1, conv_1), ...]; odd tile -> ONE custom DVE
  op in 2X_1PORT mode: each cycle port0 reads the 32b pair (rmin_i, conv_i),
  port1 reads the 32b fp32 PSUM word whose HIGH half is bf16(psum_i)
  (SRC_1_HI), and the 8-stage datapath computes
      r' = min(rmin_i, conv_i, bf16(psum_i) + l2c)
  writing (r', r') back in place. One 2258ns DVE op thus retires TWO tiles
  (vs 2258ns for ONE in v2): DVE 32x2258 = 72us, ACT 32x2000 = 64us.
  Probe-validated on HW (exact numpy match up to bf16 output rounding).
  Same-session interleaved A/B slopes (shared/noisy device, see test.py):
  v3/v2 ratio 0.74-0.95 across windows; max rel err improved 9.2e-3 ->
  8.0e-3. Engine model: 92us (v2, both engines LP-saturated) -> 72us (v3,
  DVE-bound).
  Two Z buffers (parity) keep ACT/DVE overlapped; their running mins fold in
  the tail. perf_max=1 must be set on the instruction (stock _custom_dve
  hardcodes 0, which pins custom ops to the 1x slot); the 1x slot holds a
  MAX_NEG sentinel so a silent mode fallback fails loudly in rel-err.
  bf16 (not fp16) everywhere on the consumer side: the PSUM high-half trick
  IS bf16 truncation. Adds ~0.1-0.4% rel err on top of v2's fp8-input
  ~0.92%; gate is 2e-2.
All input DMAs on the sync-engine HWDGE queue only (splitting onto the ACT
queue serialized the pipeline -- v2 finding). Dummy matmuls burn the HAM
cold-clock window during the DMA head (v2 finding).
"""

import numpy as np

N, M, D = 16384, 8192, 256
CORES = 8
C_SHIFT = 256.0

_COMPILED = {}

# --- custom DVE op MIN3_PB_ANT (see module docstring) ---------------------- #

_MIN3_NAME = "MIN3_PB_ANT"


def _min3_reference(in0, in1, s0, s1, imm2):
    """CoreSim/interp semantics: in0 = interleaved (rmin, conv) pairs; in1 =
    bf16 bitcast of the fp32 PSUM tile (odd elements = bf16 truncation);
    out pair <- (r', r') with r' = min(rmin, conv, bf16(psum) + s0)."""
    x = np.asarray(in0, np.float32)
    p = np.asarray(in1, np.float32)
    P = x.shape[0]
    x2 = x.reshape(P, -1, 2)
    bias = np.asarray(s0, np.float32).reshape(-1, 1)
    r = np.minimum(np.minimum(x2[:, :, 0], x2[:, :, 1]),
                   p.reshape(P, -1, 2)[:, :, 1] + bias)
    out = np.empty_like(x2)
    out[:, :, 0] = r
    out[:, :, 1] = r
    return out.reshape(x.shape)


def _min4_reference(in0, in1, s0, s1, imm2):
    """in0 = (rmin, convA) pairs; in1 = (convB, convC) pairs;
    out pair <- (r', r') with r' = min of all four."""
    x = np.asarray(in0, np.float32)
    y = np.asarray(in1, np.float32)
    P = x.shape[0]
    x2 = x.reshape(P, -1, 2)
    y2 = y.reshape(P, -1, 2)
    r = np.minimum(np.minimum(x2[:, :, 0], x2[:, :, 1]),
                   np.minimum(y2[:, :, 0], y2[:, :, 1]))
    out = np.empty_like(x2)
    out[:, :, 0] = r
    out[:, :, 1] = r
    return out.reshape(x.shape)


def _register_min3():
    """Register MIN3_PB_ANT + MIN4_S_ANT in concourse.dve_ops; idempotent."""
    import concourse.dve_ops as dve_ops
    from concourse.dve_spec import C0, Spec, Src0, Src1, minn
    from concourse.dve_uop import (
        ENABLE,
        AluInp,
        AluOp,
        DveOpSpec,
        InpSel,
        OutPath,
        OutSel,
        Trigger,
        UopConfig,
    )

    for op in dve_ops.OPS:
        if op.name == _MIN3_NAME:
            return op, dve_ops._MIN4_OP

    def pair_uop():
        u = UopConfig()
        u.enable_input(InpSel.SRC_0, 1)       # chain0: rmin_i
        u.enable_input(InpSel.SRC_1_HI, 2)    # chain1: bf16(psum_i)
        u.enable_input(InpSel.CONST_0, 3)     # chain2: bias
        u.enable_input(InpSel.SRC_0_HI, 4)    # chain3: conv_i
        b = u.datapath_config
        b[0].enable_alu(AluOp.ADD, AluInp.PREV_DELAY_1, AluInp.PREV_DELAY_2)
        b[0].pass_through_delay(0, 3)
        b[1].enable_alu(AluOp.MIN, AluInp.PREV_DELAY_0, AluInp.PREV_ALU_OUT)
        b[1].pass_through_delay(3)
        b[2].enable_alu(AluOp.MIN, AluInp.PREV_DELAY_3, AluInp.PREV_ALU_OUT)
        for k in range(3, 8):
            b[k].pass_through_alu()
        u.require_inp0 = ENABLE
        u.require_inp1 = ENABLE
        u.trigger = (Trigger.SRC_TENSOR_DONE, Trigger.NONE, Trigger.NONE)
        u.enable_output(OutSel.ALU_OUT, OutPath.WR0_LO)
        u.enable_output(OutSel.ALU_OUT, OutPath.WR0_HI)
        return u

    def sentinel_uop():
        # 1x slot: write MAX_NEG so a silent fallback out of 2X mode is
        # unmistakable (output collapses to 0 distances -> rel err ~1).
        u = UopConfig()
        u.enable_input(InpSel.MAX_NEG, 1)
        b = u.datapath_config
        b[0].enable_alu(AluOp.BYPASS, AluInp.PREV_DELAY_0, AluInp.PREV_DELAY_0)
        for k in range(1, 8):
            b[k].pass_through_alu()
        u.require_inp0 = ENABLE
        u.require_inp1 = ENABLE
        u.trigger = (Trigger.SRC_TENSOR_DONE, Trigger.NONE, Trigger.NONE)
        u.enable_output(OutSel.ALU_OUT, OutPath.WR0_LO)
        return u

    def min4_uop():
        # min(rmin, convA, convB, convC): in0 pairs (rmin, convA) via
        # SRC_0/SRC_0_HI, in1 pairs (convB, convC) via SRC_1/SRC_1_HI.
        # Biases were already folded by each ACT conv; no scalar needed.
        u = UopConfig()
        u.enable_input(InpSel.SRC_0, 1)       # chain0: rmin_i
        u.enable_input(InpSel.SRC_1, 2)       # chain1: convB_i
        u.enable_input(InpSel.SRC_1_HI, 3)    # chain2: convC_i
        u.enable_input(InpSel.SRC_0_HI, 4)    # chain3: convA_i
        b = u.datapath_config
        b[0].enable_alu(AluOp.MIN, AluInp.PREV_DELAY_1, AluInp.PREV_DELAY_2)
        b[0].pass_through_delay(0, 3)
        b[1].enable_alu(AluOp.MIN, AluInp.PREV_DELAY_0, AluInp.PREV_ALU_OUT)
        b[1].pass_through_delay(3)
        b[2].enable_alu(AluOp.MIN, AluInp.PREV_DELAY_3, AluInp.PREV_ALU_OUT)
        for k in range(3, 8):
            b[k].pass_through_alu()
        u.require_inp0 = ENABLE
        u.require_inp1 = ENABLE
        u.trigger = (Trigger.SRC_TENSOR_DONE, Trigger.NONE, Trigger.NONE)
        u.enable_output(OutSel.ALU_OUT, OutPath.WR0_LO)
        u.enable_output(OutSel.ALU_OUT, OutPath.WR0_HI)
        return u

    def make(name, uop2x, reference):
        row = dve_ops._CUSTOM_DVE_ROW_BASE + len(dve_ops.OPS)
        assert row < 0x20
        spec_obj = DveOpSpec(
            name=name,
            opcode=row,
            uops=[sentinel_uop()],
            uops_2x=[uop2x],
            perf_max=1,
            rd1_en=True,
        )

        class _HandOp:
            pass

        _HandOp.name = name
        _HandOp.spec = Spec(body=minn(minn(Src0, Src1), C0),
                            reference=reference)
        _HandOp.subdim = False
        _HandOp.perf_en = {}
        _HandOp.compile = lambda self, ver, _s=spec_obj: _s
        op = _HandOp()
        dve_ops.OPS.append(op)
        dve_ops.CUSTOM_DVE_SPECS[name] = op.spec
        dve_ops._SUB_OPCODE_FOR_NAME[name] = row
        return op

    op3 = make(_MIN3_NAME, pair_uop(), _min3_reference)
    op4 = make("MIN4_S_ANT", min4_uop(), _min4_reference)
    dve_ops._MIN4_OP = op4
    return op3, op4


def _emit_min3(nc, out, in0, in1, s0, which: int = 0):
    """Emit MIN3_PB_ANT (which=0) or MIN4_S_ANT (which=1) with perf_max=1
    (2X slot reachable; stock _custom_dve hardcodes perf_max=0 which pins
    custom ops to 1x)."""
    import concourse.bass_isa as bass_isa
    import concourse.dve_ops as dve_ops
    from concourse import mybir

    op = _register_min3()[which]
    v = nc.vector
    bass = v.bass
    if op.name not in bass.m.ant_custom_dve_ops:
        bass.m.ant_custom_dve_ops = sorted(
            {*bass.m.ant_custom_dve_ops, op.name})
    shape = bass_isa.CustomDveShape.TTSS
    isa_opcode = bass.isa.Opcode[
        f"NEURON_ISA_TPB_OPCODE_CUSTOM_DVE_ANT_{shape.slot()}"
    ].value
    s0_arg = (mybir.ImmediateValue(dtype=mybir.dt.float32, value=float(s0))
              if isinstance(s0, (int, float)) else v.lower_ap(s0, for_isa=True))
    ins = [
        v.lower_ap(in0, for_isa=True, opt=True),
        v.lower_ap(in1, for_isa=True, opt=True),
        s0_arg,
        mybir.ImmediateValue(dtype=mybir.dt.float32, value=0.0),
    ]
    outs = [v.lower_ap(out, for_isa=True, opt=True)]
    return v.add_instruction(
        bass_isa.InstCustomDveAnt(
            name=bass.get_next_instruction_name(),
            op_name=op.name,
            rd1_en=True,
            subdim=0,
            imm2=0.0,
            shape=shape,
            row=dve_ops.get_dve_sub_opcode(op.name),
            isa_opcode=isa_opcode,
            perf_max=1,
            ins=ins,
            outs=outs,
        )
    )


def _build(ucols: int, m: int, pattern=None, debug: bool = False, rounds: int = 1,
           mm_mode: str = "drswi", **_ignored):
    """Build + compile the per-core Bass kernel.

    ucols:  number of U columns (rows of U_z) this core handles.
    m:      number of L rows (library size).
    rounds: repeat the whole computation this many times inside a hardware
            loop (benchmarking only -- slope between round counts isolates
            steady-state HW time from the host dispatch overhead).
    """
    from contextlib import ExitStack, nullcontext

    import concourse.bacc as bacc
    import concourse.tile as tile
    from concourse import mybir

    F32 = mybir.dt.float32
    BF16 = mybir.dt.bfloat16
    FP8 = mybir.dt.float8e4
    AF = mybir.ActivationFunctionType
    ALU = mybir.AluOpType
    DR = (mybir.MatmulPerfMode.DoubleRowSwInterleave if mm_mode == "drswi"
          else mybir.MatmulPerfMode.DoubleRow)

    ltiles = m // 128
    assert ucols % 512 == 0 and m % 128 == 0
    assert ltiles % 4 == 0

    nc = bacc.Bacc("TRN2", target_bir_lowering=False, debug=debug)

    blocks = ucols // 32
    ut_d = nc.dram_tensor("ut", [128, 2, ucols], FP8, kind="ExternalInput").ap()
    lt_shape = [128, 2 * m] if mm_mode == "drswi" else [128, 2, m]
    lt_d = nc.dram_tensor("lt", lt_shape, FP8, kind="ExternalInput").ap()
    l2c_d = nc.dram_tensor("l2c", [128, ltiles], F32, kind="ExternalInput").ap()
    u2c_d = nc.dram_tensor("u2c", [32, blocks], F32, kind="ExternalInput").ap()
    out_d = nc.dram_tensor("out", [32, blocks], F32, kind="ExternalOutput").ap()

    with tile.TileContext(nc) as tc, ExitStack() as ctx:
        const_pool = ctx.enter_context(tc.tile_pool(name="const", bufs=1))
        psum_pool = ctx.enter_context(
            tc.tile_pool(name="psum", bufs=2, space="PSUM"))

        ut_sb = const_pool.tile([128, 2, ucols], FP8, name="utsb")
        lt_sb = const_pool.tile(lt_shape, FP8, name="ltsb")
        l2c = const_pool.tile([128, ltiles], F32, name="l2c")
        u2c = const_pool.tile([32, blocks], F32, name="u2c")
        # Interleaved (running-min, conv-staging) pair buffers; two for
        # ACT/DVE overlap (per-Z serial chain conv -> min3 -> conv ...).
        z0 = const_pool.tile([128, 2 * ucols], BF16, name="z0")
        z1 = const_pool.tile([128, 2 * ucols], BF16, name="z1")
        zs = (z0, z1)
        zviews = tuple(z.rearrange("p (n two) -> p n two", two=2) for z in zs)
        # Staging for the min4 ('S') units: (convB, convC) interleaved.
        w = const_pool.tile([128, 2 * ucols], BF16, name="w")
        wview = w.rearrange("p (n two) -> p n two", two=2)

        loop_cm = tc.For_i(0, rounds, 1) if rounds > 1 else nullcontext()
        ctx.enter_context(loop_cm)

        # Small + U loads first so the main loop can start on L-chunk 0.
        # ut is split so the first matmul (needs ut[:, :, 0:512] only) waits
        # on 128KB, not the full 512KB.
        nc.sync.dma_start(l2c[:], l2c_d[:])
        nc.sync.dma_start(u2c[:], u2c_d[:])
        for c0 in range(0, ucols, 512):
            nc.sync.dma_start(ut_sb[:, :, c0:c0 + 512],
                              ut_d[:, :, c0:c0 + 512])
        if mm_mode == "drswi":
            CH = min(2048, 2 * m)
            for c0 in range(0, 2 * m, CH):
                nc.sync.dma_start(lt_sb[:, c0:c0 + CH], lt_d[:, c0:c0 + CH])
        else:
            CH = min(1024, m)
            for c0 in range(0, m, CH):
                nc.sync.dma_start(lt_sb[:, :, c0:c0 + CH], lt_d[:, :, c0:c0 + CH])

        # Dummy matmuls during the DMA head: burn the HAM cold-clock window
        # (PE at 1.2 GHz until ~3.4us of sustained activity) on scratch
        # weights so the real tiles start at 2.4 GHz.
        wght = const_pool.tile([128, 256], FP8, name="wght")
        wsrc = const_pool.tile([128, 2, 512], FP8, name="wsrc")
        nc.vector.memset(wght.bitcast(F32)[:], 1.0)
        nc.vector.memset(wsrc.bitcast(F32)[:], 1.0)
        wpsum = psum_pool.tile([128, ucols], F32, name="psum", tag="psum")
        for _ in range(8):
            nc.tensor.matmul(wpsum[:, 0:512], wght[:], wsrc[:],
                             start=True, stop=True, perf_mode=DR)

        # Pool-engine memsets: the DVE is the bottleneck engine; Pool is idle
        # (COPY/MEMSET/TENSOR_SCALAR are the only legal Pool opcodes on V3).
        nc.gpsimd.memset(z0[:], 30000.0)
        nc.gpsimd.memset(z1[:], 30000.0)

        def mm_tile(lt):
            psum = psum_pool.tile([128, ucols], F32, name="psum", tag="psum")
            if mm_mode == "drswi":
                lhsT = lt_sb[:, lt * 256:(lt + 1) * 256]
            else:
                lhsT = lt_sb[:, :, lt * 128:(lt + 1) * 128]
            for s0 in range(0, ucols, 512):
                nc.tensor.matmul(
                    psum[:, s0:s0 + 512],
                    lhsT,
                    ut_sb[:, :, s0:s0 + 512],
                    start=True,
                    stop=True,
                    perf_mode=DR,
                )
            return psum

        # Schedule: 'P' = conv tile + min3 tile (2 tiles, 1 DVE op);
        # 'S' = 3 conv tiles + one min4 (3 tiles, 1 DVE op). For 64 tiles:
        # 29 P + 2 S -> ACT 35 convs (70us) vs DVE 31 ops (70us), balanced
        # (vs 32/32 = 64/72.3 DVE-bound).
        if ltiles == 64:
            sched = ["P"] * 10 + ["S"] + ["P"] * 10 + ["S"] + ["P"] * 9
        else:
            assert ltiles % 2 == 0
            sched = ["P"] * (ltiles // 2)
        lt = 0
        zi = 0
        for unit in sched:
            if unit == "P":
                # ACT: conv = bf16(psum + l2c) into the odd (staging) slots.
                psum = mm_tile(lt)
                nc.scalar.activation(zviews[zi][:, :, 1], psum[:],
                                     AF.Identity, bias=l2c[:, lt:lt + 1],
                                     scale=1.0)
                lt += 1
                # Custom DVE op: one 2X pass retires this PSUM tile AND the
                # staged conv: rmin = min(rmin, conv, bf16(psum) + l2c).
                psum = mm_tile(lt)
                _emit_min3(nc, zs[zi][:], zs[zi][:],
                           psum.bitcast(BF16)[:], l2c[:, lt:lt + 1])
                lt += 1
            else:
                # 3 convs (A -> Z odd slots, B/C -> W even/odd), then one
                # min4: rmin = min(rmin, convA, convB, convC).
                for dst in (zviews[zi][:, :, 1], wview[:, :, 0],
                            wview[:, :, 1]):
                    psum = mm_tile(lt)
                    nc.scalar.activation(dst, psum[:], AF.Identity,
                                         bias=l2c[:, lt:lt + 1], scale=1.0)
                    lt += 1
                _emit_min3(nc, zs[zi][:], zs[zi][:], w[:], 0.0, which=1)
            zi ^= 1
        assert lt == ltiles

        # Fold the two Z chains' running mins (even slots) -> contiguous.
        rmin = const_pool.tile([128, ucols], BF16, name="rmin")
        nc.vector.tensor_tensor(rmin[:], zviews[0][:, :, 0],
                                zviews[1][:, :, 0], op=ALU.min)

        # Partition reduction: transpose every 32x32 block, min over the
        # free dim within each block -> red[32g + i, b] = min over
        # partitions {32g..32g+31} of column 32b + i. Then two tree levels
        # across the four partition groups (base partitions must be
        # 32-aligned and equal for DVE TT, so realign with tiny DMAs).
        tr = const_pool.tile([128, ucols], BF16, name="tr")
        nc.vector.transpose(tr[:], rmin[:])
        red = const_pool.tile([128, blocks], BF16, name="red")
        nc.vector.tensor_reduce(
            red[:], tr.rearrange("p (b j) -> p b j", j=32),
            axis=mybir.AxisListType.X, op=ALU.min,
        )
        half = const_pool.tile([64, blocks], BF16, name="half")
        nc.sync.dma_start(half[:], red[64:128, :])
        nc.vector.tensor_tensor(red[:64, :], red[:64, :], half[:, :], op=ALU.min)
        quart = const_pool.tile([32, blocks], BF16, name="quart")
        nc.sync.dma_start(quart[:], red[32:64, :])
        nc.vector.tensor_tensor(red[:32, :], red[:32, :], quart[:, :], op=ALU.min)
        pmin = red[:32, :]
        d2 = const_pool.tile([32, blocks], F32, name="d2")
        nc.vector.tensor_tensor(d2[:], pmin[:], u2c[:], op=ALU.add)
        nc.vector.tensor_scalar_max(d2[:], d2[:], 0.0)
        outt = const_pool.tile([32, blocks], F32, name="outt")
        nc.scalar.activation(outt[:], d2[:], AF.Sqrt)
        nc.sync.dma_start(out_d[:], outt[:])

    nc.compile()
    return nc


def _get_compiled(ucols: int, m: int):
    key = (ucols, m)
    if key not in _COMPILED:
        _COMPILED[key] = _build(ucols, m)
    return _COMPILED[key]


def _prep_inputs(U: np.ndarray, L: np.ndarray, mm_mode: str = "drswi"):
    """Host-side sharding / layout prep (transpose, -2 scale, norm rows).

    Moving operand (U) DoubleRow layout: tile[p, i, x] = T[i*128 + p, x]
    for the transposed operand T [256, X] (logical K index = i*128 + p).
    Stationary operand (L) for DoubleRowSwInterleave: per L-tile, 256
    bytes per partition with w[p, 2*j + i] = LT[i*128 + p, tile*128 +
    (127 - j)] (pairs interleaved per column, columns reversed), so the
    hardware LDWEIGHTS is a contiguous read.
    """
    import ml_dtypes

    n, d = U.shape
    m = L.shape[0]
    ucols = n // CORES
    FP8 = ml_dtypes.float8_e4m3
    UTm2 = np.ascontiguousarray((-2.0 * U).T).reshape(2, 128, n)
    UTm2 = UTm2.transpose(1, 0, 2)  # [128, 2, n]
    LT3 = np.ascontiguousarray(L.T).reshape(2, 128, m)  # [i, p, dcol]
    if mm_mode == "drswi":
        # [i, p, tile, j'] with column reversal inside each 128-wide tile
        B = LT3.reshape(2, 128, m // 128, 128)[:, :, :, ::-1]
        # -> [p, tile, j', i] -> flatten to [128, 2*m]
        LT8 = np.ascontiguousarray(
            B.transpose(1, 2, 3, 0).reshape(128, 2 * m)).astype(FP8)
    else:
        LT8 = np.ascontiguousarray(LT3.transpose(1, 0, 2)).astype(FP8)
    l2 = (L.astype(np.float64) ** 2).sum(1).astype(np.float32)
    u2 = (U.astype(np.float64) ** 2).sum(1).astype(np.float32)
    l2cT = np.ascontiguousarray((l2 - C_SHIFT).reshape(m // 128, 128).T)
    u2c = u2 + C_SHIFT
    in_maps = []
    for i in range(CORES):
        sl = slice(i * ucols, (i + 1) * ucols)
        # Device output layout is [32, ucols//32] with column c = 32*b + i at
        # [i, b]; u2c must match that layout.
        u2c_dev = np.ascontiguousarray(u2c[sl].reshape(ucols // 32, 32).T)
        in_maps.append({
            "ut": np.ascontiguousarray(UTm2[:, :, sl]).astype(FP8),
            "lt": LT8,
            "l2c": l2cT,
            "u2c": u2c_dev,
        })
    return in_maps


def kernel(**inputs) -> np.ndarray:
    from concourse import bass_utils

    U = np.asarray(inputs["U_z"], dtype=np.float32)
    L = np.asarray(inputs["L_z"], dtype=np.float32)
    n = U.shape[0]
    m = L.shape[0]
    ucols = n // CORES
    nc = _get_compiled(ucols, m)
    in_maps = _prep_inputs(U, L)
    res = bass_utils.run_bass_kernel_spmd(nc, in_maps, list(range(CORES)))
    # Per-core output [32, ucols//32] holds column c = 32*b + i at [i, b].
    return np.concatenate(
        [np.ascontiguousarray(r["out"].T).reshape(-1) for r in res.results]
    ).astype(np.float32)


if __name__ == "__main__":
    # Smoke test with random data against a numpy reference.
    rng = np.random.default_rng(0)
    U = rng.standard_normal((N, D), dtype=np.float32)
    L = rng.standard_normal((M, D), dtype=np.float32)
    out = kernel(pred=None, U_z=U, L_z=L)
    d2 = (U * U).sum(1)[:, None] + (L * L).sum(1)[None, :] - 2.0 * U @ L.T
    exp = np.sqrt(np.maximum(d2, 0.0).min(1))
    rel = np.abs(out - exp) / np.maximum(np.abs(exp), 1e-9)
    print("max rel err:", rel.max())


# revision 17
# speedup vs baseline: 1.0333x; 1.0333x over previous
"""Trainium2 Bass kernel for batched nearest-neighbor min-distance.

Problem: for each row u of U_z [16384, 256], compute
    min_{l in L_z [8192, 256]} ||u - l||_2
Strategy (8 NeuronCores, data-parallel over rows of U_z, L_z replicated;
`pred` is unused by the reference and ignored):
  d2(u,l) = ||u||^2 + ||l||^2 - 2 u.l
v3: fp8(e4m3) DoubleRowSwInterleave matmuls (as v2) + a CUSTOM DVE uop
(MIN3_PB_ANT, registered at runtime into concourse.dve_ops) that breaks the
two-engine consumer bound:
  Per core (2048 U cols), 64 L-tiles of [128 Lrows x 2048 Ucols] fp32 PSUM.
  v2 consumed each tile with either an ACT conv (2.0us) + DVE fp16 merge
  (1.13us) or a DVE fused stt (2.26us); LP-optimal mix ~92us/core of engine
  time (both engines saturated -- measured ~90-115us slope).
  v3 consumes tiles in PAIRS: even tile -> ACT conv (bias=l2c, fp32 PSUM ->
  bf16) written STRIDE-2 into the odd slots of an interleaved buffer
  Z = [(rmin_0, conv_0), (rmin_1, conv_1), ...]; odd tile -> ONE custom DVE
  op in 2X_1PORT mode: each cycle port0 reads the 32b pair (rmin_i, conv_i),
  port1 reads the 32b fp32 PSUM word whose HIGH half is bf16(psum_i)
  (SRC_1_HI), and the 8-stage datapath computes
      r' = min(rmin_i, conv_i, bf16(psum_i) + l2c)
  writing (r', r') back in place. One 2258ns DVE op thus retires TWO tiles
  (vs 2258ns for ONE in v2): DVE 32x2258 = 72us, ACT 32x2000 = 64us.
  Probe-validated on HW (exact numpy match up to bf16 output rounding).
  Same-session interleaved A/B slopes (shared/noisy device, see test.py):
  v3/v2 ratio 0.74-0.95 across windows; max rel err improved 9.2e-3 ->
  8.0e-3. Engine model: 92us (v2, both engines LP-saturated) -> 72us (v3,
  DVE-bound).
  Two Z buffers (parity) keep ACT/DVE overlapped; their running mins fold in
  the tail. perf_max=1 must be set on the instruction (stock _custom_dve
  hardcodes 0, which pins custom ops to the 1x slot); the 1x slot holds a
  MAX_NEG sentinel so a silent mode fallback fails loudly in rel-err.
  bf16 (not fp16) everywhere on the consumer side: the PSUM high-half trick
  IS bf16 truncation. Adds ~0.1-0.4% rel err on top of v2's fp8-input
  ~0.92%; gate is 2e-2.
v4 on top of v3: (a) MIN4_S_ANT, a second custom uop (all-SBUF min4:
in0=(rmin,convA) pairs, in1=(convB,convC) pairs) -- schedule 29 'P' units
(conv tile + min3 tile) + 2 'S' units (3 conv tiles + one min4) rebalances
ACT/DVE engine time from 64/72.3us to 70/70us; (b) Z-init memsets moved to
the idle Pool engine (-2.25us DVE/round); (c) the ut head DMA split 4-way so
the first matmul waits on 128KB, not 512KB (-8-10us single-shot head).
Same-session A/B at 1025 rounds could not separate v3/v4 (deltas within the
shared-device noise); the engine model favors v4 and its exact build was
verified end-to-end (max rel err 8.179e-3, gate 2e-2).
All input DMAs on the sync-engine HWDGE queue only (splitting onto the ACT
queue serialized the pipeline -- v2 finding). Dummy matmuls burn the HAM
cold-clock window during the DMA head (v2 finding).
"""

import numpy as np

N, M, D = 16384, 8192, 256
CORES = 8
C_SHIFT = 256.0

_COMPILED = {}

# --- custom DVE op MIN3_PB_ANT (see module docstring) ---------------------- #

_MIN3_NAME = "MIN3_PB_ANT"


def _min3_reference(in0, in1, s0, s1, imm2):
    """CoreSim/interp semantics: in0 = interleaved (rmin, conv) pairs; in1 =
    bf16 bitcast of the fp32 PSUM tile (odd elements = bf16 truncation);
    out pair <- (r', r') with r' = min(rmin, conv, bf16(psum) + s0)."""
    x = np.asarray(in0, np.float32)
    p = np.asarray(in1, np.float32)
    P = x.shape[0]
    x2 = x.reshape(P, -1, 2)
    bias = np.asarray(s0, np.float32).reshape(-1, 1)
    r = np.minimum(np.minimum(x2[:, :, 0], x2[:, :, 1]),
                   p.reshape(P, -1, 2)[:, :, 1] + bias)
    out = np.empty_like(x2)
    out[:, :, 0] = r
    out[:, :, 1] = r
    return out.reshape(x.shape)


def _min4_reference(in0, in1, s0, s1, imm2):
    """in0 = (rmin, convA) pairs; in1 = (convB, convC) pairs;
    out pair <- (r', r') with r' = min of all four."""
    x = np.asarray(in0, np.float32)
    y = np.asarray(in1, np.float32)
    P = x.shape[0]
    x2 = x.reshape(P, -1, 2)
    y2 = y.reshape(P, -1, 2)
    r = np.minimum(np.minimum(x2[:, :, 0], x2[:, :, 1]),
                   np.minimum(y2[:, :, 0], y2[:, :, 1]))
    out = np.empty_like(x2)
    out[:, :, 0] = r
    out[:, :, 1] = r
    return out.reshape(x.shape)


def _register_min3():
    """Register MIN3_PB_ANT + MIN4_S_ANT in concourse.dve_ops; idempotent."""
    import concourse.dve_ops as dve_ops
    from concourse.dve_spec import C0, Spec, Src0, Src1, minn
    from concourse.dve_uop import (
        ENABLE,
        AluInp,
        AluOp,
        DveOpSpec,
        InpSel,
        OutPath,
        OutSel,
        Trigger,
        UopConfig,
    )

    for op in dve_ops.OPS:
        if op.name == _MIN3_NAME:
            return op, dve_ops._MIN4_OP

    def pair_uop():
        u = UopConfig()
        u.enable_input(InpSel.SRC_0, 1)       # chain0: rmin_i
        u.enable_input(InpSel.SRC_1_HI, 2)    # chain1: bf16(psum_i)
        u.enable_input(InpSel.CONST_0, 3)     # chain2: bias
        u.enable_input(InpSel.SRC_0_HI, 4)    # chain3: conv_i
        b = u.datapath_config
        b[0].enable_alu(AluOp.ADD, AluInp.PREV_DELAY_1, AluInp.PREV_DELAY_2)
        b[0].pass_through_delay(0, 3)
        b[1].enable_alu(AluOp.MIN, AluInp.PREV_DELAY_0, AluInp.PREV_ALU_OUT)
        b[1].pass_through_delay(3)
        b[2].enable_alu(AluOp.MIN, AluInp.PREV_DELAY_3, AluInp.PREV_ALU_OUT)
        for k in range(3, 8):
            b[k].pass_through_alu()
        u.require_inp0 = ENABLE
        u.require_inp1 = ENABLE
        u.trigger = (Trigger.SRC_TENSOR_DONE, Trigger.NONE, Trigger.NONE)
        u.enable_output(OutSel.ALU_OUT, OutPath.WR0_LO)
        u.enable_output(OutSel.ALU_OUT, OutPath.WR0_HI)
        return u

    def sentinel_uop():
        # 1x slot: write MAX_NEG so a silent fallback out of 2X mode is
        # unmistakable (output collapses to 0 distances -> rel err ~1).
        u = UopConfig()
        u.enable_input(InpSel.MAX_NEG, 1)
        b = u.datapath_config
        b[0].enable_alu(AluOp.BYPASS, AluInp.PREV_DELAY_0, AluInp.PREV_DELAY_0)
        for k in range(1, 8):
            b[k].pass_through_alu()
        u.require_inp0 = ENABLE
        u.require_inp1 = ENABLE
        u.trigger = (Trigger.SRC_TENSOR_DONE, Trigger.NONE, Trigger.NONE)
        u.enable_output(OutSel.ALU_OUT, OutPath.WR0_LO)
        return u

    def min4_uop():
        # min(rmin, convA, convB, convC): in0 pairs (rmin, convA) via
        # SRC_0/SRC_0_HI, in1 pairs (convB, convC) via SRC_1/SRC_1_HI.
        # Biases were already folded by each ACT conv; no scalar needed.
        u = UopConfig()
        u.enable_input(InpSel.SRC_0, 1)       # chain0: rmin_i
        u.enable_input(InpSel.SRC_1, 2)       # chain1: convB_i
        u.enable_input(InpSel.SRC_1_HI, 3)    # chain2: convC_i
        u.enable_input(InpSel.SRC_0_HI, 4)    # chain3: convA_i
        b = u.datapath_config
        b[0].enable_alu(AluOp.MIN, AluInp.PREV_DELAY_1, AluInp.PREV_DELAY_2)
        b[0].pass_through_delay(0, 3)
        b[1].enable_alu(AluOp.MIN, AluInp.PREV_DELAY_0, AluInp.PREV_ALU_OUT)
        b[1].pass_through_delay(3)
        b[2].enable_alu(AluOp.MIN, AluInp.PREV_DELAY_3, AluInp.PREV_ALU_OUT)
        for k in range(3, 8):
            b[k].pass_through_alu()
        u.require_inp0 = ENABLE
        u.require_inp1 = ENABLE
        u.trigger = (Trigger.SRC_TENSOR_DONE, Trigger.NONE, Trigger.NONE)
        u.enable_output(OutSel.ALU_OUT, OutPath.WR0_LO)
        u.enable_output(OutSel.ALU_OUT, OutPath.WR0_HI)
        return u

    def make(name, uop2x, reference):
        row = dve_ops._CUSTOM_DVE_ROW_BASE + len(dve_ops.OPS)
        assert row < 0x20
        spec_obj = DveOpSpec(
            name=name,
            opcode=row,
            uops=[sentinel_uop()],
            uops_2x=[uop2x],
            perf_max=1,
            rd1_en=True,
        )

        class _HandOp:
            pass

        _HandOp.name = name
        _HandOp.spec = Spec(body=minn(minn(Src0, Src1), C0),
                            reference=reference)
        _HandOp.subdim = False
        _HandOp.perf_en = {}
        _HandOp.compile = lambda self, ver, _s=spec_obj: _s
        op = _HandOp()
        dve_ops.OPS.append(op)
        dve_ops.CUSTOM_DVE_SPECS[name] = op.spec
        dve_ops._SUB_OPCODE_FOR_NAME[name] = row
        return op

    op3 = make(_MIN3_NAME, pair_uop(), _min3_reference)
    op4 = make("MIN4_S_ANT", min4_uop(), _min4_reference)
    dve_ops._MIN4_OP = op4
    return op3, op4


def _emit_min3(nc, out, in0, in1, s0, which: int = 0):
    """Emit MIN3_PB_ANT (which=0) or MIN4_S_ANT (which=1) with perf_max=1
    (2X slot reachable; stock _custom_dve hardcodes perf_max=0 which pins
    custom ops to 1x)."""
    import concourse.bass_isa as bass_isa
    import concourse.dve_ops as dve_ops
    from concourse import mybir

    op = _register_min3()[which]
    v = nc.vector
    bass = v.bass
    if op.name not in bass.m.ant_custom_dve_ops:
        bass.m.ant_custom_dve_ops = sorted(
            {*bass.m.ant_custom_dve_ops, op.name})
    shape = bass_isa.CustomDveShape.TTSS
    isa_opcode = bass.isa.Opcode[
        f"NEURON_ISA_TPB_OPCODE_CUSTOM_DVE_ANT_{shape.slot()}"
    ].value
    s0_arg = (mybir.ImmediateValue(dtype=mybir.dt.float32, value=float(s0))
              if isinstance(s0, (int, float)) else v.lower_ap(s0, for_isa=True))
    ins = [
        v.lower_ap(in0, for_isa=True, opt=True),
        v.lower_ap(in1, for_isa=True, opt=True),
        s0_arg,
        mybir.ImmediateValue(dtype=mybir.dt.float32, value=0.0),
    ]
    outs = [v.lower_ap(out, for_isa=True, opt=True)]
    return v.add_instruction(
        bass_isa.InstCustomDveAnt(
            name=bass.get_next_instruction_name(),
            op_name=op.name,
            rd1_en=True,
            subdim=0,
            imm2=0.0,
            shape=shape,
            row=dve_ops.get_dve_sub_opcode(op.name),
            isa_opcode=isa_opcode,
            perf_max=1,
            ins=ins,
            outs=outs,
        )
    )


def _build(ucols: int, m: int, pattern=None, debug: bool = False, rounds: int = 1,
           mm_mode: str = "drswi", use_min4: bool = True, gps_memset: bool = True,
           ut_split: bool = True, **_ignored):
    """Build + compile the per-core Bass kernel.

    ucols:  number of U columns (rows of U_z) this core handles.
    m:      number of L rows (library size).
    rounds: repeat the whole computation this many times inside a hardware
            loop (benchmarking only -- slope between round counts isolates
            steady-state HW time from the host dispatch overhead).
    """
    from contextlib import ExitStack, nullcontext

    import concourse.bacc as bacc
    import concourse.tile as tile
    from concourse import mybir

    F32 = mybir.dt.float32
    BF16 = mybir.dt.bfloat16
    FP8 = mybir.dt.float8e4
    AF = mybir.ActivationFunctionType
    ALU = mybir.AluOpType
    DR = (mybir.MatmulPerfMode.DoubleRowSwInterleave if mm_mode == "drswi"
          else mybir.MatmulPerfMode.DoubleRow)

    ltiles = m // 128
    assert ucols % 512 == 0 and m % 128 == 0
    assert ltiles % 4 == 0

    nc = bacc.Bacc("TRN2", target_bir_lowering=False, debug=debug)

    blocks = ucols // 32
    ut_d = nc.dram_tensor("ut", [128, 2, ucols], FP8, kind="ExternalInput").ap()
    lt_shape = [128, 2 * m] if mm_mode == "drswi" else [128, 2, m]
    lt_d = nc.dram_tensor("lt", lt_shape, FP8, kind="ExternalInput").ap()
    l2c_d = nc.dram_tensor("l2c", [128, ltiles], F32, kind="ExternalInput").ap()
    u2c_d = nc.dram_tensor("u2c", [32, blocks], F32, kind="ExternalInput").ap()
    out_d = nc.dram_tensor("out", [32, blocks], F32, kind="ExternalOutput").ap()

    with tile.TileContext(nc) as tc, ExitStack() as ctx:
        const_pool = ctx.enter_context(tc.tile_pool(name="const", bufs=1))
        psum_pool = ctx.enter_context(
            tc.tile_pool(name="psum", bufs=2, space="PSUM"))

        ut_sb = const_pool.tile([128, 2, ucols], FP8, name="utsb")
        lt_sb = const_pool.tile(lt_shape, FP8, name="ltsb")
        l2c = const_pool.tile([128, ltiles], F32, name="l2c")
        u2c = const_pool.tile([32, blocks], F32, name="u2c")
        # Interleaved (running-min, conv-staging) pair buffers; two for
        # ACT/DVE overlap (per-Z serial chain conv -> min3 -> conv ...).
        z0 = const_pool.tile([128, 2 * ucols], BF16, name="z0")
        z1 = const_pool.tile([128, 2 * ucols], BF16, name="z1")
        zs = (z0, z1)
        zviews = tuple(z.rearrange("p (n two) -> p n two", two=2) for z in zs)
        # Staging for the min4 ('S') units: (convB, convC) interleaved.
        w = const_pool.tile([128, 2 * ucols], BF16, name="w")
        wview = w.rearrange("p (n two) -> p n two", two=2)

        loop_cm = tc.For_i(0, rounds, 1) if rounds > 1 else nullcontext()
        ctx.enter_context(loop_cm)

        # Small + U loads first so the main loop can start on L-chunk 0.
        # ut is split so the first matmul (needs ut[:, :, 0:512] only) waits
        # on 128KB, not the full 512KB.
        nc.sync.dma_start(l2c[:], l2c_d[:])
        nc.sync.dma_start(u2c[:], u2c_d[:])
        if ut_split:
            for c0 in range(0, ucols, 512):
                nc.sync.dma_start(ut_sb[:, :, c0:c0 + 512],
                                  ut_d[:, :, c0:c0 + 512])
        else:
            nc.sync.dma_start(ut_sb[:], ut_d[:])
        if mm_mode == "drswi":
            CH = min(2048, 2 * m)
            for c0 in range(0, 2 * m, CH):
                nc.sync.dma_start(lt_sb[:, c0:c0 + CH], lt_d[:, c0:c0 + CH])
        else:
            CH = min(1024, m)
            for c0 in range(0, m, CH):
                nc.sync.dma_start(lt_sb[:, :, c0:c0 + CH], lt_d[:, :, c0:c0 + CH])

        # Dummy matmuls during the DMA head: burn the HAM cold-clock window
        # (PE at 1.2 GHz until ~3.4us of sustained activity) on scratch
        # weights so the real tiles start at 2.4 GHz.
        wght = const_pool.tile([128, 256], FP8, name="wght")
        wsrc = const_pool.tile([128, 2, 512], FP8, name="wsrc")
        nc.vector.memset(wght.bitcast(F32)[:], 1.0)
        nc.vector.memset(wsrc.bitcast(F32)[:], 1.0)
        wpsum = psum_pool.tile([128, ucols], F32, name="psum", tag="psum")
        for _ in range(8):
            nc.tensor.matmul(wpsum[:, 0:512], wght[:], wsrc[:],
                             start=True, stop=True, perf_mode=DR)

        # Pool-engine memsets: the DVE is the bottleneck engine; Pool is idle
        # (COPY/MEMSET/TENSOR_SCALAR are the only legal Pool opcodes on V3).
        ms_eng = nc.gpsimd if gps_memset else nc.vector
        ms_eng.memset(z0[:], 30000.0)
        ms_eng.memset(z1[:], 30000.0)

        def mm_tile(lt):
            psum = psum_pool.tile([128, ucols], F32, name="psum", tag="psum")
            if mm_mode == "drswi":
                lhsT = lt_sb[:, lt * 256:(lt + 1) * 256]
            else:
                lhsT = lt_sb[:, :, lt * 128:(lt + 1) * 128]
            for s0 in range(0, ucols, 512):
                nc.tensor.matmul(
                    psum[:, s0:s0 + 512],
                    lhsT,
                    ut_sb[:, :, s0:s0 + 512],
                    start=True,
                    stop=True,
                    perf_mode=DR,
                )
            return psum

        # Schedule: 'P' = conv tile + min3 tile (2 tiles, 1 DVE op);
        # 'S' = 3 conv tiles + one min4 (3 tiles, 1 DVE op). For 64 tiles:
        # 29 P + 2 S -> ACT 35 convs (70us) vs DVE 31 ops (70us), balanced
        # (vs 32/32 = 64/72.3 DVE-bound).
        if ltiles == 64 and use_min4:
            sched = ["P"] * 10 + ["S"] + ["P"] * 10 + ["S"] + ["P"] * 9
        else:
            assert ltiles % 2 == 0
            sched = ["P"] * (ltiles // 2)
        lt = 0
        zi = 0
        for unit in sched:
            if unit == "P":
                # ACT: conv = bf16(psum + l2c) into the odd (staging) slots.
                psum = mm_tile(lt)
                nc.scalar.activation(zviews[zi][:, :, 1], psum[:],
                                     AF.Identity, bias=l2c[:, lt:lt + 1],
                                     scale=1.0)
                lt += 1
                # Custom DVE op: one 2X pass retires this PSUM tile AND the
                # staged conv: rmin = min(rmin, conv, bf16(psum) + l2c).
                psum = mm_tile(lt)
                _emit_min3(nc, zs[zi][:], zs[zi][:],
                           psum.bitcast(BF16)[:], l2c[:, lt:lt + 1])
                lt += 1
            else:
                # 3 convs (A -> Z odd slots, B/C -> W even/odd), then one
                # min4: rmin = min(rmin, convA, convB, convC).
                for dst in (zviews[zi][:, :, 1], wview[:, :, 0],
                            wview[:, :, 1]):
                    psum = mm_tile(lt)
                    nc.scalar.activation(dst, psum[:], AF.Identity,
                                         bias=l2c[:, lt:lt + 1], scale=1.0)
                    lt += 1
                _emit_min3(nc, zs[zi][:], zs[zi][:], w[:], 0.0, which=1)
            zi ^= 1
        assert lt == ltiles

        # Fold the two Z chains' running mins (even slots) -> contiguous.
        rmin = const_pool.tile([128, ucols], BF16, name="rmin")
        nc.vector.tensor_tensor(rmin[:], zviews[0][:, :, 0],
                                zviews[1][:, :, 0], op=ALU.min)

        # Partition reduction: transpose every 32x32 block, min over the
        # free dim within each block -> red[32g + i, b] = min over
        # partitions {32g..32g+31} of column 32b + i. Then two tree levels
        # across the four partition groups (base partitions must be
        # 32-aligned and equal for DVE TT, so realign with tiny DMAs).
        tr = const_pool.tile([128, ucols], BF16, name="tr")
        nc.vector.transpose(tr[:], rmin[:])
        red = const_pool.tile([128, blocks], BF16, name="red")
        nc.vector.tensor_reduce(
            red[:], tr.rearrange("p (b j) -> p b j", j=32),
            axis=mybir.AxisListType.X, op=ALU.min,
        )
        # Partition-group tree: realign groups 1..3 onto partitions 0:32 with
        # three INDEPENDENT DMAs issued in parallel (the old half/quart chain
        # serialized DMA latency behind each TT), then three tiny TT mins.
        ga = const_pool.tile([32, blocks], BF16, name="ga")
        gb = const_pool.tile([32, blocks], BF16, name="gb")
        gc = const_pool.tile([32, blocks], BF16, name="gc")
        nc.sync.dma_start(ga[:], red[32:64, :])
        nc.sync.dma_start(gb[:], red[64:96, :])
        nc.sync.dma_start(gc[:], red[96:128, :])
        nc.vector.tensor_tensor(red[:32, :], red[:32, :], ga[:, :], op=ALU.min)
        nc.vector.tensor_tensor(red[:32, :], red[:32, :], gb[:, :], op=ALU.min)
        nc.vector.tensor_tensor(red[:32, :], red[:32, :], gc[:, :], op=ALU.min)
        pmin = red[:32, :]
        d2 = const_pool.tile([32, blocks], F32, name="d2")
        nc.vector.tensor_tensor(d2[:], pmin[:], u2c[:], op=ALU.add)
        nc.vector.tensor_scalar_max(d2[:], d2[:], 0.0)
        outt = const_pool.tile([32, blocks], F32, name="outt")
        nc.scalar.activation(outt[:], d2[:], AF.Sqrt)
        nc.sync.dma_start(out_d[:], outt[:])

    nc.compile()
    return nc


def _get_compiled(ucols: int, m: int):
    key = (ucols, m)
    if key not in _COMPILED:
        _COMPILED[key] = _build(ucols, m)
    return _COMPILED[key]


def _prep_inputs(U: np.ndarray, L: np.ndarray, mm_mode: str = "drswi"):
    """Host-side sharding / layout prep (transpose, -2 scale, norm rows).

    Moving operand (U) DoubleRow layout: tile[p, i, x] = T[i*128 + p, x]
    for the transposed operand T [256, X] (logical K index = i*128 + p).
    Stationary operand (L) for DoubleRowSwInterleave: per L-tile, 256
    bytes per partition with w[p, 2*j + i] = LT[i*128 + p, tile*128 +
    (127 - j)] (pairs interleaved per column, columns reversed), so the
    hardware LDWEIGHTS is a contiguous read.
    """
    import ml_dtypes

    n, d = U.shape
    m = L.shape[0]
    ucols = n // CORES
    FP8 = ml_dtypes.float8_e4m3
    UTm2 = np.ascontiguousarray((-2.0 * U).T).reshape(2, 128, n)
    UTm2 = UTm2.transpose(1, 0, 2)  # [128, 2, n]
    LT3 = np.ascontiguousarray(L.T).reshape(2, 128, m)  # [i, p, dcol]
    if mm_mode == "drswi":
        # [i, p, tile, j'] with column reversal inside each 128-wide tile
        B = LT3.reshape(2, 128, m // 128, 128)[:, :, :, ::-1]
        # -> [p, tile, j', i] -> flatten to [128, 2*m]
        LT8 = np.ascontiguousarray(
            B.transpose(1, 2, 3, 0).reshape(128, 2 * m)).astype(FP8)
    else:
        LT8 = np.ascontiguousarray(LT3.transpose(1, 0, 2)).astype(FP8)
    l2 = (L.astype(np.float64) ** 2).sum(1).astype(np.float32)
    u2 = (U.astype(np.float64) ** 2).sum(1).astype(np.float32)
    l2cT = np.ascontiguousarray((l2 - C_SHIFT).reshape(m // 128, 128).T)
    u2c = u2 + C_SHIFT
    in_maps = []
    for i in range(CORES):
        sl = slice(i * ucols, (i + 1) * ucols)
        # Device output layout is [32, ucols//32] with column c = 32*b + i at
        # [i, b]; u2c must match that layout.
        u2c_dev = np.ascontiguousarray(u2c[sl].reshape(ucols // 32, 32).T)
        in_maps.append({
            "ut": np.ascontiguousarray(UTm2[:, :, sl]).astype(FP8),
            "lt": LT8,
            "l2c": l2cT,
            "u2c": u2c_dev,
        })
    return in_maps


def kernel(**inputs) -> np.ndarray:
    from concourse import bass_utils

    U = np.asarray(inputs["U_z"], dtype=np.float32)
    L = np.asarray(inputs["L_z"], dtype=np.float32)
    n = U.shape[0]
    m = L.shape[0]
    ucols = n // CORES
    nc = _get_compiled(ucols, m)
    in_maps = _prep_inputs(U, L)
    res = bass_utils.run_bass_kernel_spmd(nc, in_maps, list(range(CORES)))
    # Per-core output [32, ucols//32] holds column c = 32*b + i at [i, b].
    return np.concatenate(
        [np.ascontiguousarray(r["out"].T).reshape(-1) for r in res.results]
    ).astype(np.float32)


if __name__ == "__main__":
    # Smoke test with random data against a numpy reference.
    rng = np.random.default_rng(0)
    U = rng.standard_normal((N, D), dtype=np.float32)
    L = rng.standard_normal((M, D), dtype=np.float32)
    out = kernel(pred=None, U_z=U, L_z=L)
    d2 = (U * U).sum(1)[:, None] + (L * L).sum(1)[None, :] - 2.0 * U @ L.T
    exp = np.sqrt(np.maximum(d2, 0.0).min(1))
    rel = np.abs(out - exp) / np.maximum(np.abs(exp), 1e-9)
    print("max rel err:", rel.max())


# revision 18
# speedup vs baseline: 1.0711x; 1.0365x over previous
"""Trainium2 Bass kernel for batched nearest-neighbor min-distance.

Problem: for each row u of U_z [16384, 256], compute
    min_{l in L_z [8192, 256]} ||u - l||_2
Strategy (8 NeuronCores, data-parallel over rows of U_z, L_z replicated;
`pred` is unused by the reference and ignored):
  d2(u,l) = ||u||^2 + ||l||^2 - 2 u.l
v3: fp8(e4m3) DoubleRowSwInterleave matmuls (as v2) + a CUSTOM DVE uop
(MIN3_PB_ANT, registered at runtime into concourse.dve_ops) that breaks the
two-engine consumer bound:
  Per core (2048 U cols), 64 L-tiles of [128 Lrows x 2048 Ucols] fp32 PSUM.
  v2 consumed each tile with either an ACT conv (2.0us) + DVE fp16 merge
  (1.13us) or a DVE fused stt (2.26us); LP-optimal mix ~92us/core of engine
  time (both engines saturated -- measured ~90-115us slope).
  v3 consumes tiles in PAIRS: even tile -> ACT conv (bias=l2c, fp32 PSUM ->
  bf16) written STRIDE-2 into the odd slots of an interleaved buffer
  Z = [(rmin_0, conv_0), (rmin_1, conv_1), ...]; odd tile -> ONE custom DVE
  op in 2X_1PORT mode: each cycle port0 reads the 32b pair (rmin_i, conv_i),
  port1 reads the 32b fp32 PSUM word whose HIGH half is bf16(psum_i)
  (SRC_1_HI), and the 8-stage datapath computes
      r' = min(rmin_i, conv_i, bf16(psum_i) + l2c)
  writing (r', r') back in place. One 2258ns DVE op thus retires TWO tiles
  (vs 2258ns for ONE in v2): DVE 32x2258 = 72us, ACT 32x2000 = 64us.
  Probe-validated on HW (exact numpy match up to bf16 output rounding).
  Same-session interleaved A/B slopes (shared/noisy device, see test.py):
  v3/v2 ratio 0.74-0.95 across windows; max rel err improved 9.2e-3 ->
  8.0e-3. Engine model: 92us (v2, both engines LP-saturated) -> 72us (v3,
  DVE-bound).
  Two Z buffers (parity) keep ACT/DVE overlapped; their running mins fold in
  the tail. perf_max=1 must be set on the instruction (stock _custom_dve
  hardcodes 0, which pins custom ops to the 1x slot); the 1x slot holds a
  MAX_NEG sentinel so a silent mode fallback fails loudly in rel-err.
  bf16 (not fp16) everywhere on the consumer side: the PSUM high-half trick
  IS bf16 truncation. Adds ~0.1-0.4% rel err on top of v2's fp8-input
  ~0.92%; gate is 2e-2.
v4 on top of v3: (a) MIN4_S_ANT, a second custom uop (all-SBUF min4:
in0=(rmin,convA) pairs, in1=(convB,convC) pairs) -- schedule 29 'P' units
(conv tile + min3 tile) + 2 'S' units (3 conv tiles + one min4) rebalances
ACT/DVE engine time from 64/72.3us to 70/70us; (b) Z-init memsets moved to
the idle Pool engine (-2.25us DVE/round); (c) the ut head DMA split 4-way so
the first matmul waits on 128KB, not 512KB (-8-10us single-shot head).
Same-session A/B at 1025 rounds could not separate v3/v4 (deltas within the
shared-device noise); the engine model favors v4 and its exact build was
verified end-to-end (max rel err 8.179e-3, gate 2e-2).
All input DMAs on the sync-engine HWDGE queue only (splitting onto the ACT
queue serialized the pipeline -- v2 finding). Dummy matmuls burn the HAM
cold-clock window during the DMA head (v2 finding).
"""

import numpy as np

N, M, D = 16384, 8192, 256
CORES = 8
C_SHIFT = 256.0

_COMPILED = {}

# --- custom DVE op MIN3_PB_ANT (see module docstring) ---------------------- #

_MIN3_NAME = "MIN3_PB_ANT"


def _min3_reference(in0, in1, s0, s1, imm2):
    """CoreSim/interp semantics: in0 = interleaved (rmin, conv) pairs; in1 =
    bf16 bitcast of the fp32 PSUM tile (odd elements = bf16 truncation);
    out pair <- (r', r') with r' = min(rmin, conv, bf16(psum) + s0)."""
    x = np.asarray(in0, np.float32)
    p = np.asarray(in1, np.float32)
    P = x.shape[0]
    x2 = x.reshape(P, -1, 2)
    bias = np.asarray(s0, np.float32).reshape(-1, 1)
    r = np.minimum(np.minimum(x2[:, :, 0], x2[:, :, 1]),
                   p.reshape(P, -1, 2)[:, :, 1] + bias)
    out = np.empty_like(x2)
    out[:, :, 0] = r
    out[:, :, 1] = r
    return out.reshape(x.shape)


def _min4_reference(in0, in1, s0, s1, imm2):
    """in0 = (rmin, convA) pairs; in1 = (convB, convC) pairs;
    out pair <- (r', r') with r' = min of all four."""
    x = np.asarray(in0, np.float32)
    y = np.asarray(in1, np.float32)
    P = x.shape[0]
    x2 = x.reshape(P, -1, 2)
    y2 = y.reshape(P, -1, 2)
    r = np.minimum(np.minimum(x2[:, :, 0], x2[:, :, 1]),
                   np.minimum(y2[:, :, 0], y2[:, :, 1]))
    out = np.empty_like(x2)
    out[:, :, 0] = r
    out[:, :, 1] = r
    return out.reshape(x.shape)


def _register_min3():
    """Register MIN3_PB_ANT + MIN4_S_ANT in concourse.dve_ops; idempotent."""
    import concourse.dve_ops as dve_ops
    from concourse.dve_spec import C0, Spec, Src0, Src1, minn
    from concourse.dve_uop import (
        ENABLE,
        AluInp,
        AluOp,
        DveOpSpec,
        InpSel,
        OutPath,
        OutSel,
        Trigger,
        UopConfig,
    )

    for op in dve_ops.OPS:
        if op.name == _MIN3_NAME:
            return op, dve_ops._MIN4_OP

    def pair_uop():
        u = UopConfig()
        u.enable_input(InpSel.SRC_0, 1)       # chain0: rmin_i
        u.enable_input(InpSel.SRC_1_HI, 2)    # chain1: bf16(psum_i)
        u.enable_input(InpSel.CONST_0, 3)     # chain2: bias
        u.enable_input(InpSel.SRC_0_HI, 4)    # chain3: conv_i
        b = u.datapath_config
        b[0].enable_alu(AluOp.ADD, AluInp.PREV_DELAY_1, AluInp.PREV_DELAY_2)
        b[0].pass_through_delay(0, 3)
        b[1].enable_alu(AluOp.MIN, AluInp.PREV_DELAY_0, AluInp.PREV_ALU_OUT)
        b[1].pass_through_delay(3)
        b[2].enable_alu(AluOp.MIN, AluInp.PREV_DELAY_3, AluInp.PREV_ALU_OUT)
        for k in range(3, 8):
            b[k].pass_through_alu()
        u.require_inp0 = ENABLE
        u.require_inp1 = ENABLE
        u.trigger = (Trigger.SRC_TENSOR_DONE, Trigger.NONE, Trigger.NONE)
        u.enable_output(OutSel.ALU_OUT, OutPath.WR0_LO)
        u.enable_output(OutSel.ALU_OUT, OutPath.WR0_HI)
        return u

    def sentinel_uop():
        # 1x slot: write MAX_NEG so a silent fallback out of 2X mode is
        # unmistakable (output collapses to 0 distances -> rel err ~1).
        u = UopConfig()
        u.enable_input(InpSel.MAX_NEG, 1)
        b = u.datapath_config
        b[0].enable_alu(AluOp.BYPASS, AluInp.PREV_DELAY_0, AluInp.PREV_DELAY_0)
        for k in range(1, 8):
            b[k].pass_through_alu()
        u.require_inp0 = ENABLE
        u.require_inp1 = ENABLE
        u.trigger = (Trigger.SRC_TENSOR_DONE, Trigger.NONE, Trigger.NONE)
        u.enable_output(OutSel.ALU_OUT, OutPath.WR0_LO)
        return u

    def min4_uop():
        # min(rmin, convA, convB, convC): in0 pairs (rmin, convA) via
        # SRC_0/SRC_0_HI, in1 pairs (convB, convC) via SRC_1/SRC_1_HI.
        # Biases were already folded by each ACT conv; no scalar needed.
        u = UopConfig()
        u.enable_input(InpSel.SRC_0, 1)       # chain0: rmin_i
        u.enable_input(InpSel.SRC_1, 2)       # chain1: convB_i
        u.enable_input(InpSel.SRC_1_HI, 3)    # chain2: convC_i
        u.enable_input(InpSel.SRC_0_HI, 4)    # chain3: convA_i
        b = u.datapath_config
        b[0].enable_alu(AluOp.MIN, AluInp.PREV_DELAY_1, AluInp.PREV_DELAY_2)
        b[0].pass_through_delay(0, 3)
        b[1].enable_alu(AluOp.MIN, AluInp.PREV_DELAY_0, AluInp.PREV_ALU_OUT)
        b[1].pass_through_delay(3)
        b[2].enable_alu(AluOp.MIN, AluInp.PREV_DELAY_3, AluInp.PREV_ALU_OUT)
        for k in range(3, 8):
            b[k].pass_through_alu()
        u.require_inp0 = ENABLE
        u.require_inp1 = ENABLE
        u.trigger = (Trigger.SRC_TENSOR_DONE, Trigger.NONE, Trigger.NONE)
        u.enable_output(OutSel.ALU_OUT, OutPath.WR0_LO)
        u.enable_output(OutSel.ALU_OUT, OutPath.WR0_HI)
        return u

    def make(name, uop2x, reference):
        row = dve_ops._CUSTOM_DVE_ROW_BASE + len(dve_ops.OPS)
        assert row < 0x20
        spec_obj = DveOpSpec(
            name=name,
            opcode=row,
            uops=[sentinel_uop()],
            uops_2x=[uop2x],
            perf_max=1,
            rd1_en=True,
        )

        class _HandOp:
            pass

        _HandOp.name = name
        _HandOp.spec = Spec(body=minn(minn(Src0, Src1), C0),
                            reference=reference)
        _HandOp.subdim = False
        _HandOp.perf_en = {}
        _HandOp.compile = lambda self, ver, _s=spec_obj: _s
        op = _HandOp()
        dve_ops.OPS.append(op)
        dve_ops.CUSTOM_DVE_SPECS[name] = op.spec
        dve_ops._SUB_OPCODE_FOR_NAME[name] = row
        return op

    op3 = make(_MIN3_NAME, pair_uop(), _min3_reference)
    op4 = make("MIN4_S_ANT", min4_uop(), _min4_reference)
    dve_ops._MIN4_OP = op4
    return op3, op4


def _emit_min3(nc, out, in0, in1, s0, which: int = 0):
    """Emit MIN3_PB_ANT (which=0) or MIN4_S_ANT (which=1) with perf_max=1
    (2X slot reachable; stock _custom_dve hardcodes perf_max=0 which pins
    custom ops to 1x)."""
    import concourse.bass_isa as bass_isa
    import concourse.dve_ops as dve_ops
    from concourse import mybir

    op = _register_min3()[which]
    v = nc.vector
    bass = v.bass
    if op.name not in bass.m.ant_custom_dve_ops:
        bass.m.ant_custom_dve_ops = sorted(
            {*bass.m.ant_custom_dve_ops, op.name})
    shape = bass_isa.CustomDveShape.TTSS
    isa_opcode = bass.isa.Opcode[
        f"NEURON_ISA_TPB_OPCODE_CUSTOM_DVE_ANT_{shape.slot()}"
    ].value
    s0_arg = (mybir.ImmediateValue(dtype=mybir.dt.float32, value=float(s0))
              if isinstance(s0, (int, float)) else v.lower_ap(s0, for_isa=True))
    ins = [
        v.lower_ap(in0, for_isa=True, opt=True),
        v.lower_ap(in1, for_isa=True, opt=True),
        s0_arg,
        mybir.ImmediateValue(dtype=mybir.dt.float32, value=0.0),
    ]
    outs = [v.lower_ap(out, for_isa=True, opt=True)]
    return v.add_instruction(
        bass_isa.InstCustomDveAnt(
            name=bass.get_next_instruction_name(),
            op_name=op.name,
            rd1_en=True,
            subdim=0,
            imm2=0.0,
            shape=shape,
            row=dve_ops.get_dve_sub_opcode(op.name),
            isa_opcode=isa_opcode,
            perf_max=1,
            ins=ins,
            outs=outs,
        )
    )


def _build(ucols: int, m: int, pattern=None, debug: bool = False, rounds: int = 1,
           mm_mode: str = "drswi", use_min4: bool = True, gps_memset: bool = True,
           ut_split: bool = True, **_ignored):
    """Build + compile the per-core Bass kernel.

    ucols:  number of U columns (rows of U_z) this core handles.
    m:      number of L rows (library size).
    rounds: repeat the whole computation this many times inside a hardware
            loop (benchmarking only -- slope between round counts isolates
            steady-state HW time from the host dispatch overhead).
    """
    from contextlib import ExitStack, nullcontext

    import concourse.bacc as bacc
    import concourse.tile as tile
    from concourse import mybir

    F32 = mybir.dt.float32
    BF16 = mybir.dt.bfloat16
    FP8 = mybir.dt.float8e4
    AF = mybir.ActivationFunctionType
    ALU = mybir.AluOpType
    DR = (mybir.MatmulPerfMode.DoubleRowSwInterleave if mm_mode == "drswi"
          else mybir.MatmulPerfMode.DoubleRow)

    ltiles = m // 128
    assert ucols % 512 == 0 and m % 128 == 0
    assert ltiles % 4 == 0

    nc = bacc.Bacc("TRN2", target_bir_lowering=False, debug=debug)

    blocks = ucols // 32
    ut_d = nc.dram_tensor("ut", [128, 2, ucols], FP8, kind="ExternalInput").ap()
    lt_shape = [128, 2 * m] if mm_mode == "drswi" else [128, 2, m]
    lt_d = nc.dram_tensor("lt", lt_shape, FP8, kind="ExternalInput").ap()
    l2c_d = nc.dram_tensor("l2c", [128, ltiles], F32, kind="ExternalInput").ap()
    u2c_d = nc.dram_tensor("u2c", [32, blocks], F32, kind="ExternalInput").ap()
    out_d = nc.dram_tensor("out", [32, blocks], F32, kind="ExternalOutput").ap()

    with tile.TileContext(nc) as tc, ExitStack() as ctx:
        const_pool = ctx.enter_context(tc.tile_pool(name="const", bufs=1))
        psum_pool = ctx.enter_context(
            tc.tile_pool(name="psum", bufs=2, space="PSUM"))

        ut_sb = const_pool.tile([128, 2, ucols], FP8, name="utsb")
        lt_sb = const_pool.tile(lt_shape, FP8, name="ltsb")
        l2c = const_pool.tile([128, ltiles], F32, name="l2c")
        u2c = const_pool.tile([32, blocks], F32, name="u2c")
        # Interleaved (running-min, conv-staging) pair buffers; two for
        # ACT/DVE overlap (per-Z serial chain conv -> min3 -> conv ...).
        z0 = const_pool.tile([128, 2 * ucols], BF16, name="z0")
        z1 = const_pool.tile([128, 2 * ucols], BF16, name="z1")
        zs = (z0, z1)
        zviews = tuple(z.rearrange("p (n two) -> p n two", two=2) for z in zs)
        # Staging for the min4 ('S') units: (convB, convC) interleaved.
        w = const_pool.tile([128, 2 * ucols], BF16, name="w")
        wview = w.rearrange("p (n two) -> p n two", two=2)

        # Warmup-matmul scratch (round-invariant constants; hoisted out of
        # the rounds loop so the DVE never re-initializes them).
        wght = const_pool.tile([128, 256], FP8, name="wght")
        wsrc = const_pool.tile([128, 2, 512], FP8, name="wsrc")
        nc.vector.memset(wght.bitcast(F32)[:], 1.0)
        nc.vector.memset(wsrc.bitcast(F32)[:], 1.0)

        loop_cm = tc.For_i(0, rounds, 1) if rounds > 1 else nullcontext()
        ctx.enter_context(loop_cm)

        # Small + U loads first so the main loop can start on L-chunk 0.
        # ut is split so the first matmul (needs ut[:, :, 0:512] only) waits
        # on 128KB, not the full 512KB.
        nc.sync.dma_start(l2c[:], l2c_d[:])
        nc.sync.dma_start(u2c[:], u2c_d[:])
        if ut_split:
            for c0 in range(0, ucols, 512):
                nc.sync.dma_start(ut_sb[:, :, c0:c0 + 512],
                                  ut_d[:, :, c0:c0 + 512])
        else:
            nc.sync.dma_start(ut_sb[:], ut_d[:])
        if mm_mode == "drswi":
            CH = min(2048, 2 * m)
            for c0 in range(0, 2 * m, CH):
                nc.sync.dma_start(lt_sb[:, c0:c0 + CH], lt_d[:, c0:c0 + CH])
        else:
            CH = min(1024, m)
            for c0 in range(0, m, CH):
                nc.sync.dma_start(lt_sb[:, :, c0:c0 + CH], lt_d[:, :, c0:c0 + CH])

        # Dummy matmuls during the DMA head: burn the HAM cold-clock window
        # (PE at 1.2 GHz until ~3.4us of sustained activity) on scratch
        # weights so the real tiles start at 2.4 GHz.
        wpsum = psum_pool.tile([128, ucols], F32, name="psum", tag="psum")
        for _ in range(8):
            nc.tensor.matmul(wpsum[:, 0:512], wght[:], wsrc[:],
                             start=True, stop=True, perf_mode=DR)

        # Pool-engine memsets: the DVE is the bottleneck engine; Pool is idle
        # (COPY/MEMSET/TENSOR_SCALAR are the only legal Pool opcodes on V3).
        ms_eng = nc.gpsimd if gps_memset else nc.vector
        ms_eng.memset(z0[:], 30000.0)
        ms_eng.memset(z1[:], 30000.0)

        def mm_tile(lt):
            psum = psum_pool.tile([128, ucols], F32, name="psum", tag="psum")
            if mm_mode == "drswi":
                lhsT = lt_sb[:, lt * 256:(lt + 1) * 256]
            else:
                lhsT = lt_sb[:, :, lt * 128:(lt + 1) * 128]
            for s0 in range(0, ucols, 512):
                nc.tensor.matmul(
                    psum[:, s0:s0 + 512],
                    lhsT,
                    ut_sb[:, :, s0:s0 + 512],
                    start=True,
                    stop=True,
                    perf_mode=DR,
                )
            return psum

        # Schedule: 'P' = conv tile + min3 tile (2 tiles, 1 DVE op);
        # 'S' = 3 conv tiles + one min4 (3 tiles, 1 DVE op). For 64 tiles:
        # 29 P + 2 S -> ACT 35 convs (70us) vs DVE 31 ops (70us), balanced
        # (vs 32/32 = 64/72.3 DVE-bound).
        if ltiles == 64 and use_min4:
            sched = ["P"] * 10 + ["S"] + ["P"] * 10 + ["S"] + ["P"] * 9
        else:
            assert ltiles % 2 == 0
            sched = ["P"] * (ltiles // 2)
        lt = 0
        zi = 0
        for unit in sched:
            if unit == "P":
                # ACT: conv = bf16(psum + l2c) into the odd (staging) slots.
                psum = mm_tile(lt)
                nc.scalar.activation(zviews[zi][:, :, 1], psum[:],
                                     AF.Identity, bias=l2c[:, lt:lt + 1],
                                     scale=1.0)
                lt += 1
                # Custom DVE op: one 2X pass retires this PSUM tile AND the
                # staged conv: rmin = min(rmin, conv, bf16(psum) + l2c).
                psum = mm_tile(lt)
                _emit_min3(nc, zs[zi][:], zs[zi][:],
                           psum.bitcast(BF16)[:], l2c[:, lt:lt + 1])
                lt += 1
            else:
                # 3 convs (A -> Z odd slots, B/C -> W even/odd), then one
                # min4: rmin = min(rmin, convA, convB, convC).
                for dst in (zviews[zi][:, :, 1], wview[:, :, 0],
                            wview[:, :, 1]):
                    psum = mm_tile(lt)
                    nc.scalar.activation(dst, psum[:], AF.Identity,
                                         bias=l2c[:, lt:lt + 1], scale=1.0)
                    lt += 1
                _emit_min3(nc, zs[zi][:], zs[zi][:], w[:], 0.0, which=1)
            zi ^= 1
        assert lt == ltiles

        # Fold the two Z chains' running mins (even slots) -> contiguous.
        rmin = const_pool.tile([128, ucols], BF16, name="rmin")
        nc.vector.tensor_tensor(rmin[:], zviews[0][:, :, 0],
                                zviews[1][:, :, 0], op=ALU.min)

        # Partition reduction: transpose every 32x32 block, min over the
        # free dim within each block -> red[32g + i, b] = min over
        # partitions {32g..32g+31} of column 32b + i. Then two tree levels
        # across the four partition groups (base partitions must be
        # 32-aligned and equal for DVE TT, so realign with tiny DMAs).
        tr = const_pool.tile([128, ucols], BF16, name="tr")
        nc.vector.transpose(tr[:], rmin[:])
        red = const_pool.tile([128, blocks], BF16, name="red")
        nc.vector.tensor_reduce(
            red[:], tr.rearrange("p (b j) -> p b j", j=32),
            axis=mybir.AxisListType.X, op=ALU.min,
        )
        # Partition-group tree: realign groups 1..3 onto partitions 0:32 with
        # three INDEPENDENT DMAs issued in parallel (the old half/quart chain
        # serialized DMA latency behind each TT), then three tiny TT mins.
        ga = const_pool.tile([32, blocks], BF16, name="ga")
        gb = const_pool.tile([32, blocks], BF16, name="gb")
        gc = const_pool.tile([32, blocks], BF16, name="gc")
        nc.sync.dma_start(ga[:], red[32:64, :])
        nc.sync.dma_start(gb[:], red[64:96, :])
        nc.sync.dma_start(gc[:], red[96:128, :])
        nc.vector.tensor_tensor(red[:32, :], red[:32, :], ga[:, :], op=ALU.min)
        nc.vector.tensor_tensor(red[:32, :], red[:32, :], gb[:, :], op=ALU.min)
        nc.vector.tensor_tensor(red[:32, :], red[:32, :], gc[:, :], op=ALU.min)
        pmin = red[:32, :]
        d2 = const_pool.tile([32, blocks], F32, name="d2")
        nc.vector.tensor_tensor(d2[:], pmin[:], u2c[:], op=ALU.add)
        nc.vector.tensor_scalar_max(d2[:], d2[:], 0.0)
        outt = const_pool.tile([32, blocks], F32, name="outt")
        nc.scalar.activation(outt[:], d2[:], AF.Sqrt)
        nc.sync.dma_start(out_d[:], outt[:])

    nc.compile()
    return nc


def _get_compiled(ucols: int, m: int):
    key = (ucols, m)
    if key not in _COMPILED:
        _COMPILED[key] = _build(ucols, m)
    return _COMPILED[key]


def _prep_inputs(U: np.ndarray, L: np.ndarray, mm_mode: str = "drswi"):
    """Host-side sharding / layout prep (transpose, -2 scale, norm rows).

    Moving operand (U) DoubleRow layout: tile[p, i, x] = T[i*128 + p, x]
    for the transposed operand T [256, X] (logical K index = i*128 + p).
    Stationary operand (L) for DoubleRowSwInterleave: per L-tile, 256
    bytes per partition with w[p, 2*j + i] = LT[i*128 + p, tile*128 +
    (127 - j)] (pairs interleaved per column, columns reversed), so the
    hardware LDWEIGHTS is a contiguous read.
    """
    import ml_dtypes

    n, d = U.shape
    m = L.shape[0]
    ucols = n // CORES
    FP8 = ml_dtypes.float8_e4m3
    UTm2 = np.ascontiguousarray((-2.0 * U).T).reshape(2, 128, n)
    UTm2 = UTm2.transpose(1, 0, 2)  # [128, 2, n]
    LT3 = np.ascontiguousarray(L.T).reshape(2, 128, m)  # [i, p, dcol]
    if mm_mode == "drswi":
        # [i, p, tile, j'] with column reversal inside each 128-wide tile
        B = LT3.reshape(2, 128, m // 128, 128)[:, :, :, ::-1]
        # -> [p, tile, j', i] -> flatten to [128, 2*m]
        LT8 = np.ascontiguousarray(
            B.transpose(1, 2, 3, 0).reshape(128, 2 * m)).astype(FP8)
    else:
        LT8 = np.ascontiguousarray(LT3.transpose(1, 0, 2)).astype(FP8)
    l2 = (L.astype(np.float64) ** 2).sum(1).astype(np.float32)
    u2 = (U.astype(np.float64) ** 2).sum(1).astype(np.float32)
    l2cT = np.ascontiguousarray((l2 - C_SHIFT).reshape(m // 128, 128).T)
    u2c = u2 + C_SHIFT
    in_maps = []
    for i in range(CORES):
        sl = slice(i * ucols, (i + 1) * ucols)
        # Device output layout is [32, ucols//32] with column c = 32*b + i at
        # [i, b]; u2c must match that layout.
        u2c_dev = np.ascontiguousarray(u2c[sl].reshape(ucols // 32, 32).T)
        in_maps.append({
            "ut": np.ascontiguousarray(UTm2[:, :, sl]).astype(FP8),
            "lt": LT8,
            "l2c": l2cT,
            "u2c": u2c_dev,
        })
    return in_maps


def kernel(**inputs) -> np.ndarray:
    from concourse import bass_utils

    U = np.asarray(inputs["U_z"], dtype=np.float32)
    L = np.asarray(inputs["L_z"], dtype=np.float32)
    n = U.shape[0]
    m = L.shape[0]
    ucols = n // CORES
    nc = _get_compiled(ucols, m)
    in_maps = _prep_inputs(U, L)
    res = bass_utils.run_bass_kernel_spmd(nc, in_maps, list(range(CORES)))
    # Per-core output [32, ucols//32] holds column c = 32*b + i at [i, b].
    return np.concatenate(
        [np.ascontiguousarray(r["out"].T).reshape(-1) for r in res.results]
    ).astype(np.float32)


if __name__ == "__main__":
    # Smoke test with random data against a numpy reference.
    rng = np.random.default_rng(0)
    U = rng.standard_normal((N, D), dtype=np.float32)
    L = rng.standard_normal((M, D), dtype=np.float32)
    out = kernel(pred=None, U_z=U, L_z=L)
    d2 = (U * U).sum(1)[:, None] + (L * L).sum(1)[None, :] - 2.0 * U @ L.T
    exp = np.sqrt(np.maximum(d2, 0.0).min(1))
    rel = np.abs(out - exp) / np.maximum(np.abs(exp), 1e-9)
    print("max rel err:", rel.max())
